# revision 33
# baseline (speedup 1.0000x reference)
"""GCNConv (PyG-style, alpha-blended residual) on 8 Trainium2 NeuronCores.

Strategy (graph/data parallel, zero collectives):
  out = a*x + (1-a)*(Ahat @ x @ W.T + b)        (aggregate-first form)
391 natural 128-dst-node groups are dealt across 8 cores x 49 slots (sorted
so the cross-core per-slot maxima stay tight). The full x table (bf16) is
resident in every core's HBM; cross-partition halo reads are local gathers.

Gather plan (the Q7/SWDGE descriptor-gen is the bottleneck at ~7.5ns/idx):
  - per (core, half): edges of all 49 slots form ONE flat position stream
    (slot k's segment is the cross-core max count, so the stream layout is
    compile-time shared). The stream is cut into ~40-chunk arenas; ONE
    dma_gather call per arena (~21 calls/core vs 98 in the per-slot scheme,
    saving ~1us fixed cost per call), round-robin over 4 SWDGE queues so
    descriptor draining never backpressures generation.
  - the table is bf16 (512B rows): halves DMA bytes (gen cost is per-idx,
    not per-byte) and keeps the single-ring drain ahead of gen.
  - slot segments may straddle chunk boundaries; boundary chunks get one
    duplicated (masked) S column per slot so each slot's matmul only picks
    up its own edges.
Aggregation: DVE builds S[e,c,i] = (iota[i]==dofoff[e,c]) * norm[e,c] in
bf16 (2x DVE rate), PE accumulates S_c^T @ Xg_c in fp32 PSUM, transposes
agg, applies folded (1-a)*W.T (f32r), and the preblended residual
(a*x + (1-a)*b, exact fp32) is added during PSUM->SBUF on DVE.
Self-loops use a persistent [128, 49, 256] bf16 slab (no gather).
Graph preprocessing (degrees, sorting, layout) is host-side numpy.
"""

import numpy as np
import ml_dtypes

import concourse.bacc as bacc
import concourse.bass as bass
import concourse.mybir as mybir
import concourse.tile as tile
from concourse.bass_utils import run_bass_kernel_spmd

N_NODES = 50000
D = 256
M_CORES = 8
P = 128
HALF = 25000
NG = (N_NODES + P - 1) // P          # 391 natural dst groups
SLOTS = (NG + M_CORES - 1) // M_CORES  # 49 slots per core
CAP = 18                             # max chunks per gather arena

F32 = mybir.dt.float32
F32R = mybir.dt.float32r
BF16 = mybir.dt.bfloat16
I16 = mybir.dt.int16

BF = ml_dtypes.bfloat16


def _pack_arenas(nmax_h):
    """Cut the flat per-half stream (slot segments of nmax_h[s] idx) into
    arenas of <= CAP chunks, each a whole number of slots. The first arena
    is small (first matmuls unblock sooner) and the last two are small
    (short drain tail). Returns (arenas, pos0, aid): arenas = list of
    dicts(first_slot, nslots, nidx, nchunks, istart); pos0[s] = in-arena
    start position of slot s."""
    # tapered caps: small first arena, CAP in the middle, small tail
    total_chunks = -(-int(nmax_h.sum()) // P)
    arenas = []
    pos0 = np.zeros(SLOTS, dtype=np.int64)
    aid = np.zeros(SLOTS, dtype=np.int64)
    cur_n = 0
    first = 0
    done_chunks = 0
    prev_ns = []
    for s in range(SLOTS):
        n = int(nmax_h[s])
        cap = CAP
        if not arenas:
            cap = 10
        elif total_chunks - done_chunks - cur_n // P < 12:
            cap = 4
        elif total_chunks - done_chunks - cur_n // P < int(1.5 * CAP):
            cap = 8
        if cur_n > 0 and -(-(cur_n + n) // P) > cap:
            arenas.append(dict(first_slot=first, nslots=s - first, nidx=cur_n,
                               nchunks=-(-cur_n // P)))
            done_chunks += arenas[-1]["nchunks"]
            first = s
            cur_n = 0
        pos0[s] = cur_n
        aid[s] = len(arenas)
        cur_n += n
    arenas.append(dict(first_slot=first, nslots=SLOTS - first, nidx=cur_n,
                       nchunks=-(-cur_n // P)))
    return arenas, pos0, aid


def _preprocess(node_features, edge_index, W, b, alpha):
    x = np.ascontiguousarray(np.asarray(node_features, dtype=np.float32))
    ei = np.asarray(edge_index)
    a = float(np.asarray(alpha).reshape(-1)[0])
    Wf = np.asarray(W, dtype=np.float32)
    bf = np.asarray(b, dtype=np.float32)

    src = ei[0].astype(np.int64)
    dst = ei[1].astype(np.int64)

    deg = (np.bincount(dst, minlength=N_NODES) + 1).astype(np.float32)
    dinv = (1.0 / np.sqrt(deg)).astype(np.float32)  # deg >= 1 (self loops)
    nrm = dinv[src] * dinv[dst]
    dinv2 = dinv * dinv

    gg = dst // P
    doff = (dst - gg * P).astype(np.float32)
    halfb = (src >= HALF).astype(np.int64)
    key = gg * 2 + halfb

    cnt = np.bincount(key, minlength=NG * 2)
    nn0 = cnt[0::2].astype(np.int64)
    nn1 = cnt[1::2].astype(np.int64)

    # deal groups into slots of 8; sort keys on raw per-half edge counts so
    # the per-slot cross-core maxes (what the gather actually pays) are tight
    best = None
    for skey in (
        np.maximum(nn0, nn1) * 4096 + nn0 + nn1,
        nn0 * 4096 + nn1,
        nn1 * 4096 + nn0,
        nn0 + nn1,
    ):
        order = np.argsort(-skey, kind="stable")
        tot = 0
        for r in range(SLOTS):
            blk = order[r * M_CORES:(r + 1) * M_CORES]
            tot += int(nn0[blk].max()) + int(nn1[blk].max())
        if best is None or tot < best[0]:
            best = (tot, order)
    order = best[1]
    assign = np.full((M_CORES, SLOTS), -1, dtype=np.int64)
    core_of = np.zeros(NG, dtype=np.int64)
    slot_of = np.zeros(NG, dtype=np.int64)
    for r in range(SLOTS):
        blk = order[r * M_CORES:(r + 1) * M_CORES]
        for c, g in enumerate(blk):
            assign[c, r] = g
            core_of[g] = c
            slot_of[g] = r

    # local search: swap a core's groups between slots when it lowers the
    # summed per-slot maxima (what the gather actually issues)
    def _slot_cost(r):
        blk = assign[:, r]
        blk = blk[blk >= 0]
        return int(nn0[blk].max()) + int(nn1[blk].max())
    cost = np.array([_slot_cost(r) for r in range(SLOTS)], dtype=np.int64)
    for _ in range(6):
        improved = False
        for c in range(M_CORES):
            for r1 in range(SLOTS):
                for r2 in range(r1 + 1, SLOTS):
                    g1, g2 = assign[c, r1], assign[c, r2]
                    if g1 < 0 or g2 < 0:
                        continue
                    assign[c, r1], assign[c, r2] = g2, g1
                    n1, n2 = _slot_cost(r1), _slot_cost(r2)
                    if n1 + n2 < cost[r1] + cost[r2]:
                        cost[r1], cost[r2] = n1, n2
                        improved = True
                    else:
                        assign[c, r1], assign[c, r2] = g1, g2
        if not improved:
            break
    for c in range(M_CORES):
        for r in range(SLOTS):
            g = int(assign[c, r])
            if g >= 0:
                core_of[g] = c
                slot_of[g] = r

    nmax = np.zeros((SLOTS, 2), dtype=np.int64)
    for r in range(SLOTS):
        blk = assign[:, r]
        blk = blk[blk >= 0]
        nmax[r, 0] = int(nn0[blk].max())
        nmax[r, 1] = int(nn1[blk].max())

    # arena packing per half (shared across cores)
    arA, pos0A, aidA = _pack_arenas(nmax[:, 0])
    arB, pos0B, aidB = _pack_arenas(nmax[:, 1])
    # flat idx offsets, each arena padded to a full last chunk: the gather
    # then writes every byte a matmul can read, so no SBUF pre-zeroing (or
    # NaN-bit hazard from uninitialized tails) exists at all
    ist = 0
    for ar in arA + arB:
        ar["istart"] = ist
        ar["nissue"] = ar["nchunks"] * P
        ist += ar["nissue"]
    ITOT = ist

    # per-slot S-column layout: [lenA cols][lenB cols][self]
    loA = np.zeros(SLOTS, dtype=np.int64)
    lenA = np.zeros(SLOTS, dtype=np.int64)
    loB = np.zeros(SLOTS, dtype=np.int64)
    lenB = np.zeros(SLOTS, dtype=np.int64)
    for s in range(SLOTS):
        for (pos0, nm, lo, ln) in ((pos0A, nmax[s, 0], loA, lenA),
                                   (pos0B, nmax[s, 1], loB, lenB)):
            p0 = int(pos0[s])
            n = int(nm)
            if n == 0:
                lo[s] = 0
                ln[s] = 0
            else:
                lo[s] = p0 // P
                ln[s] = -(-(p0 + n) // P) - p0 // P
    scol = np.zeros(SLOTS, dtype=np.int64)
    run = 0
    for s in range(SLOTS):
        scol[s] = run
        run += int(lenA[s]) + int(lenB[s]) + 1
    NCOLS = run

    # ---- per-core value arrays
    eorder = np.argsort(key, kind="stable")
    ks = key[eorder]
    ss = src[eorder]
    nn = nrm[eorder]
    do = doff[eorder]
    starts = np.concatenate([[0], np.cumsum(cnt)[:-1]])
    pos = np.arange(ks.shape[0], dtype=np.int64) - starts[ks]

    g_e = ks // 2
    h_e = ks % 2
    cr_e = core_of[g_e]
    s_e = slot_of[g_e]
    # flat position: arena istart + in-arena slot start + in-slot index
    ar_ist = np.zeros((SLOTS, 2), dtype=np.int64)
    for s in range(SLOTS):
        ar_ist[s, 0] = arA[aidA[s]]["istart"] + pos0A[s]
        ar_ist[s, 1] = arB[aidB[s]]["istart"] + pos0B[s]
    gpos = ar_ist[s_e, h_e] + pos

    idx_arr = np.zeros((M_CORES, ITOT), dtype=np.int16)
    idx_arr[cr_e, gpos] = (ss - h_e * HALF).astype(np.int16)

    # S columns: value arrays [M, NCOLS, P]
    dofcol = np.zeros((M_CORES, NCOLS, P), dtype=np.float32)
    nrmcol = np.zeros((M_CORES, NCOLS, P), dtype=np.float32)
    # in-arena position of each edge
    ipos_e = (pos0A[s_e] * (1 - h_e) + pos0B[s_e] * h_e) + pos
    chunk_e = ipos_e // P
    part_e = ipos_e % P
    colbase = scol[s_e] + np.where(h_e == 0, chunk_e - loA[s_e],
                                   lenA[s_e] + chunk_e - loB[s_e])
    dofcol[cr_e, colbase, part_e] = do
    nrmcol[cr_e, colbase, part_e] = nn

    # self columns + xself/xres slabs
    xbf = x.astype(BF)
    xself_sl = []
    for c in range(M_CORES):
        slab = np.zeros((P, SLOTS, D), dtype=BF)
        for s in range(SLOTS):
            g = int(assign[c, s])
            cself = int(scol[s] + lenA[s] + lenB[s])
            dofcol[c, cself, :] = np.arange(P, dtype=np.float32)
            if g < 0:
                continue
            lo = g * P
            hi = min(lo + P, N_NODES)
            n = hi - lo
            slab[:n, s, :] = xbf[lo:hi]
            nrmcol[c, cself, :n] = dinv2[lo:hi]
        xself_sl.append(np.ascontiguousarray(slab.reshape(P, SLOTS * D)))
    # residual is derived on-device: out = pout + a*xself + (1-a)*b.
    # xres input now carries only the (1-a)*b bias row + the alpha scalar.
    brow = np.tile(((1.0 - a) * bf)[None, :], (P, 1)).astype(np.float32)
    xres_sl = [brow for _ in range(M_CORES)]
    has_bias = bool(np.any(bf != 0.0))

    gidx = [np.tile(idx_arr[c].reshape(-1, 16).T, (8, 1)) for c in range(M_CORES)]
    nrm_in = [np.ascontiguousarray(nrmcol[c].reshape(NCOLS * P)
                                   .reshape(NCOLS, P).T).astype(BF)
              for c in range(M_CORES)]
    off_in = [np.ascontiguousarray(dofcol[c].reshape(NCOLS, P).T).astype(BF)
              for c in range(M_CORES)]

    wtp = np.ascontiguousarray(((1.0 - a) * Wf.T).astype(np.float32))
    CMAXS = int(max(int(lenA[s]) + int(lenB[s]) + 1 for s in range(SLOTS)))
    # iota2[p, (i c)] = i: S is built in [p, i, c] layout so the per-slot
    # dof/nrm operands broadcast along the middle dim (2x DVE rate vs inner)
    iota = np.tile(
        np.broadcast_to(np.arange(P, dtype=np.float32)[:, None],
                        (P, CMAXS)).reshape(1, P * CMAXS),
        (P, 1)).astype(BF)
    ident = np.eye(P, dtype=np.float32)

    meta = dict(arA=arA, arB=arB, loA=loA, lenA=lenA, loB=loB, lenB=lenB,
                scol=scol, NCOLS=NCOLS, ITOT=ITOT, aidA=aidA, aidB=aidB,
                assign=assign, CMAXS=CMAXS, alpha=a, has_bias=has_bias)
    return (xbf, gidx, nrm_in, off_in, xres_sl, xself_sl, wtp, iota, ident, meta)


def _build(meta):
    ALPHA_C = float(meta["alpha"])
    arA, arB = meta["arA"], meta["arB"]
    loA, lenA = meta["loA"], meta["lenA"]
    loB, lenB = meta["loB"], meta["lenB"]
    scol, NCOLS, ITOT = meta["scol"], meta["NCOLS"], meta["ITOT"]
    aidA, aidB = meta["aidA"], meta["aidB"]

    CMAXS = int(meta["CMAXS"])

    nc = bacc.Bacc("TRN2", debug=False, num_swdge_queues=4)

    xtab = nc.dram_tensor("xtab", [N_NODES, D], BF16, kind="ExternalInput")
    xres = nc.dram_tensor("xres", [P, D], F32, kind="ExternalInput")
    xself = nc.dram_tensor("xself", [P, SLOTS * D], BF16, kind="ExternalInput")
    gidx = nc.dram_tensor("gidx", [P, ITOT // 16], I16, kind="ExternalInput")
    nrmv = nc.dram_tensor("nrmv", [P, NCOLS], BF16, kind="ExternalInput")
    dofv = nc.dram_tensor("dofv", [P, NCOLS], BF16, kind="ExternalInput")
    wtp = nc.dram_tensor("wtp", [2 * P, D], F32R, kind="ExternalInput")
    iota = nc.dram_tensor("iota", [P, P * CMAXS], BF16, kind="ExternalInput")
    ident = nc.dram_tensor("ident", [P, P], F32, kind="ExternalInput")
    out = nc.dram_tensor("out", [SLOTS * P, D], F32, kind="ExternalOutput")

    with tile.TileContext(nc) as tc:
        with (
            tc.tile_pool(name="const", bufs=1) as cpool,
            tc.tile_pool(name="arA", bufs=6) as arA_pool,
            tc.tile_pool(name="arB", bufs=6) as arB_pool,
            tc.tile_pool(name="sel", bufs=3) as s_pool,
            tc.tile_pool(name="sb", bufs=3) as sb_pool,
            tc.tile_pool(name="io", bufs=3) as io_pool,
            tc.tile_pool(name="pagg", bufs=2, space="PSUM") as pagg_pool,
            tc.tile_pool(name="pt", bufs=2, space="PSUM") as pt_pool,
            tc.tile_pool(name="pout", bufs=2, space="PSUM") as pout_pool,
        ):
            iota_sb = cpool.tile([P, P, CMAXS], BF16)
            ident_sb = cpool.tile([P, P], F32)
            wtp0_sb = cpool.tile([P, D], F32R)
            wtp1_sb = cpool.tile([P, D], F32R)
            gidx_sb = cpool.tile([P, ITOT // 16], I16)
            nrm_sb = cpool.tile([P, NCOLS], BF16)
            dof_sb = cpool.tile([P, NCOLS], BF16)
            xself_sb = cpool.tile([P, SLOTS, D], BF16)
            g0c = None  # first-arena idx cols load first so gather 0 starts asap
            _g0 = [a for a in (meta["arA"][0], meta["arB"][0])]
            g0c = (_g0[0]["nissue"] + _g0[1]["nissue"]) // 16
            nc.scalar.dma_start(out=gidx_sb[:, 0:g0c], in_=gidx[:, 0:g0c])
            nc.sync.dma_start(out=nrm_sb[:], in_=nrmv[:])
            nc.sync.dma_start(out=dof_sb[:], in_=dofv[:])
            nc.sync.dma_start(
                out=iota_sb[:].rearrange("p i c -> p (i c)"), in_=iota[:])
            nc.sync.dma_start(out=ident_sb[:], in_=ident[:])
            nc.sync.dma_start(out=wtp0_sb[:], in_=wtp[0:P, :])
            nc.sync.dma_start(out=wtp1_sb[:], in_=wtp[P:2 * P, :])
            bias_sb = cpool.tile([P, D], F32)
            nc.sync.dma_start(out=bias_sb[:], in_=xres[:])
            nc.sync.dma_start(out=gidx_sb[:, g0c:], in_=gidx[:, g0c:])
            nc.sync.dma_start(
                out=xself_sb[:].rearrange("p s d -> p (s d)"), in_=xself[:])

            # 16-idx warmup gather: pays the ~6us ext-isa IRAM load while
            # the constant DMAs are still in flight, so the first real
            # gather starts generating immediately.
            warm_idx = cpool.tile([P, 1], I16)
            warm_out = cpool.tile([P, 1, D], BF16)
            nc.vector.memset(warm_idx[:], 0)
            nc.gpsimd.dma_gather(
                warm_out[:], xtab[0:HALF, :], warm_idx[:],
                16, 16, D, single_packet=False, queue_num=0,
            )

            qrr = [0]

            def gather_arena(pool, ar, tab_ap, tag):
                t = pool.tile([P, CAP, D], BF16, tag=tag)
                q = qrr[0] % 4
                qrr[0] += 1
                nc.gpsimd.dma_gather(
                    t[:, 0:ar["nchunks"], :], tab_ap,
                    gidx_sb[:, ar["istart"] // 16:
                            ar["istart"] // 16 + ar["nissue"] // 16],
                    ar["nissue"], ar["nissue"], D,
                    single_packet=False, queue_num=q,
                )
                return t

            tabA = xtab[0:HALF, :]
            tabB = xtab[HALF:N_NODES, :]
            curA = None
            curB = None
            for s in range(SLOTS):
                a_id, b_id = int(aidA[s]), int(aidB[s])
                if arA[a_id]["first_slot"] == s:
                    curA = gather_arena(arA_pool, arA[a_id], tabA, "xga")
                if arB[b_id]["first_slot"] == s:
                    curB = gather_arena(arB_pool, arB[b_id], tabB, "xgb")

                lA, lB = int(lenA[s]), int(lenB[s])
                ncols = lA + lB + 1
                sc = int(scol[s])
                # S in [p, i, c] layout: dof/nrm broadcast along the middle
                # dim (fast); matmul lhsT takes strided [:, :, c] slices.
                s_tile = s_pool.tile([P, P, CMAXS], BF16, tag="sel")
                dof_b = (dof_sb[:, sc:sc + ncols]
                         .rearrange("p (i c) -> p i c", i=1)
                         .to_broadcast([P, P, ncols]))
                nrm_b = (nrm_sb[:, sc:sc + ncols]
                         .rearrange("p (i c) -> p i c", i=1)
                         .to_broadcast([P, P, ncols]))
                nc.vector.tensor_tensor(
                    out=s_tile[:, :, 0:ncols], in0=iota_sb[:, :, 0:ncols],
                    in1=dof_b, op=mybir.AluOpType.is_equal,
                )
                nc.vector.tensor_tensor(
                    out=s_tile[:, :, 0:ncols], in0=s_tile[:, :, 0:ncols],
                    in1=nrm_b, op=mybir.AluOpType.mult,
                )

                pagg = pagg_pool.tile([P, D], F32)
                k = 0
                for j in range(lA):
                    nc.tensor.matmul(
                        pagg[:], lhsT=s_tile[:, :, j],
                        rhs=curA[:, int(loA[s]) + j, :],
                        start=(k == 0), stop=False)
                    k += 1
                for j in range(lB):
                    nc.tensor.matmul(
                        pagg[:], lhsT=s_tile[:, :, lA + j],
                        rhs=curB[:, int(loB[s]) + j, :],
                        start=(k == 0), stop=False)
                    k += 1
                nc.tensor.matmul(
                    pagg[:], lhsT=s_tile[:, :, lA + lB],
                    rhs=xself_sb[:, s, :], start=(k == 0), stop=True)

                agg_sb = sb_pool.tile([P, D], F32, tag="agg")
                nc.scalar.copy(agg_sb[:], pagg[:])

                aggT_sb = sb_pool.tile([P, D], F32R, tag="aggT")
                for kb in range(2):
                    pt = pt_pool.tile([P, P], F32)
                    nc.tensor.transpose(
                        pt[:], agg_sb[:, kb * P:(kb + 1) * P], ident_sb[:]
                    )
                    nc.scalar.copy(aggT_sb[:, kb * P:(kb + 1) * P], pt[:])

                pout = pout_pool.tile([P, D], F32)
                nc.tensor.matmul(
                    pout[:], lhsT=aggT_sb[:, 0:P],
                    rhs=wtp0_sb[:], start=True, stop=False,
                )
                nc.tensor.matmul(
                    pout[:], lhsT=aggT_sb[:, P:2 * P],
                    rhs=wtp1_sb[:], start=False, stop=True,
                )

                out_sb = io_pool.tile([P, D], F32, tag="out")
                # out = a*xself + pout  (alpha from bias_sb[1,0] at build is a
                # host constant folded via scalar_tensor_tensor's scalar)
                nc.vector.scalar_tensor_tensor(
                    out=out_sb[:], in0=xself_sb[:, s, :], scalar=ALPHA_C,
                    in1=pout[:], op0=mybir.AluOpType.mult,
                    op1=mybir.AluOpType.add,
                )
                if meta["has_bias"]:
                    nc.vector.tensor_tensor(
                        out=out_sb[:], in0=out_sb[:], in1=bias_sb[:],
                        op=mybir.AluOpType.add,
                    )
                nc.sync.dma_start(out=out[s * P:(s + 1) * P, :], in_=out_sb[:])

    nc.compile()
    return nc


def kernel(node_features, edge_index, W, b, alpha):
    (x, gidx, nrm_in, off_in, xres_sl, xself_sl, wtp, iota, ident, meta) = _preprocess(
        node_features, edge_index, W, b, alpha
    )
    nc = _build(meta)
    in_maps = [
        {
            "xtab": x,
            "xres": xres_sl[c],
            "xself": xself_sl[c],
            "gidx": gidx[c],
            "nrmv": nrm_in[c],
            "dofv": off_in[c],
            "wtp": wtp,
            "iota": iota,
            "ident": ident,
        }
        for c in range(M_CORES)
    ]
    res = run_bass_kernel_spmd(nc, in_maps, list(range(M_CORES)))
    assign = meta["assign"]
    outf = np.empty((N_NODES, D), dtype=np.float32)
    for c in range(M_CORES):
        slab = res.results[c]["out"]
        for s in range(SLOTS):
            g = int(assign[c, s])
            if g < 0:
                continue
            lo = g * P
            hi = min(lo + P, N_NODES)
            outf[lo:hi] = slab[s * P: s * P + hi - lo]
    return outf


# revision 34
# speedup vs baseline: 1.0207x; 1.0207x over previous
"""GCNConv (PyG-style, alpha-blended residual) on 8 Trainium2 NeuronCores.

Strategy (graph/data parallel, zero collectives):
  out = a*x + (1-a)*(Ahat @ x @ W.T + b)        (aggregate-first form)
391 natural 128-dst-node groups are dealt across 8 cores x 49 slots (sorted
so the cross-core per-slot maxima stay tight). The full x table (bf16) is
resident in every core's HBM; cross-partition halo reads are local gathers.

Gather plan (the Q7/SWDGE descriptor-gen is the bottleneck at ~7.5ns/idx):
  - per (core, half): edges of all 49 slots form ONE flat position stream
    (slot k's segment is the cross-core max count, so the stream layout is
    compile-time shared). The stream is cut into ~40-chunk arenas; ONE
    dma_gather call per arena (~21 calls/core vs 98 in the per-slot scheme,
    saving ~1us fixed cost per call), round-robin over 4 SWDGE queues so
    descriptor draining never backpressures generation.
  - the table is bf16 (512B rows): halves DMA bytes (gen cost is per-idx,
    not per-byte) and keeps the single-ring drain ahead of gen.
  - slot segments may straddle chunk boundaries; boundary chunks get one
    duplicated (masked) S column per slot so each slot's matmul only picks
    up its own edges.
Aggregation: DVE builds S[e,c,i] = (iota[i]==dofoff[e,c]) * norm[e,c] in
bf16 (2x DVE rate), PE accumulates S_c^T @ Xg_c in fp32 PSUM, transposes
agg, applies folded (1-a)*W.T (f32r), and the preblended residual
(a*x + (1-a)*b, exact fp32) is added during PSUM->SBUF on DVE.
Self-loops use a persistent [128, 49, 256] bf16 slab (no gather).
Graph preprocessing (degrees, sorting, layout) is host-side numpy.
"""

import numpy as np
import ml_dtypes

import concourse.bacc as bacc
import concourse.bass as bass
import concourse.mybir as mybir
import concourse.tile as tile
from concourse.bass_utils import run_bass_kernel_spmd

N_NODES = 50000
D = 256
M_CORES = 8
P = 128
HALF = 25000
NG = (N_NODES + P - 1) // P          # 391 natural dst groups
SLOTS = (NG + M_CORES - 1) // M_CORES  # 49 slots per core
CAP = 20                             # max chunks per gather arena

F32 = mybir.dt.float32
F32R = mybir.dt.float32r
BF16 = mybir.dt.bfloat16
I16 = mybir.dt.int16

BF = ml_dtypes.bfloat16


def _pack_arenas(nmax_h):
    """Cut the flat per-half stream (slot segments of nmax_h[s] idx) into
    arenas of <= CAP chunks, each a whole number of slots. The first arena
    is small (first matmuls unblock sooner) and the last two are small
    (short drain tail). Returns (arenas, pos0, aid): arenas = list of
    dicts(first_slot, nslots, nidx, nchunks, istart); pos0[s] = in-arena
    start position of slot s."""
    # tapered caps: small first arena, CAP in the middle, small tail
    total_chunks = -(-int(nmax_h.sum()) // P)
    arenas = []
    pos0 = np.zeros(SLOTS, dtype=np.int64)
    aid = np.zeros(SLOTS, dtype=np.int64)
    cur_n = 0
    first = 0
    done_chunks = 0
    prev_ns = []
    for s in range(SLOTS):
        n = int(nmax_h[s])
        cap = CAP
        if not arenas:
            cap = 10
        elif total_chunks - done_chunks - cur_n // P < 12:
            cap = 4
        elif total_chunks - done_chunks - cur_n // P < int(1.5 * CAP):
            cap = 8
        if cur_n > 0 and -(-(cur_n + n) // P) > cap:
            arenas.append(dict(first_slot=first, nslots=s - first, nidx=cur_n,
                               nchunks=-(-cur_n // P)))
            done_chunks += arenas[-1]["nchunks"]
            first = s
            cur_n = 0
        pos0[s] = cur_n
        aid[s] = len(arenas)
        cur_n += n
    arenas.append(dict(first_slot=first, nslots=SLOTS - first, nidx=cur_n,
                       nchunks=-(-cur_n // P)))
    return arenas, pos0, aid


def _preprocess(node_features, edge_index, W, b, alpha):
    x = np.ascontiguousarray(np.asarray(node_features, dtype=np.float32))
    ei = np.asarray(edge_index)
    a = float(np.asarray(alpha).reshape(-1)[0])
    Wf = np.asarray(W, dtype=np.float32)
    bf = np.asarray(b, dtype=np.float32)

    src = ei[0].astype(np.int64)
    dst = ei[1].astype(np.int64)

    deg = (np.bincount(dst, minlength=N_NODES) + 1).astype(np.float32)
    dinv = (1.0 / np.sqrt(deg)).astype(np.float32)  # deg >= 1 (self loops)
    nrm = dinv[src] * dinv[dst]
    dinv2 = dinv * dinv

    gg = dst // P
    doff = (dst - gg * P).astype(np.float32)
    halfb = (src >= HALF).astype(np.int64)
    key = gg * 2 + halfb

    cnt = np.bincount(key, minlength=NG * 2)
    nn0 = cnt[0::2].astype(np.int64)
    nn1 = cnt[1::2].astype(np.int64)

    # deal groups into slots of 8; sort keys on raw per-half edge counts so
    # the per-slot cross-core maxes (what the gather actually pays) are tight
    best = None
    for skey in (
        np.maximum(nn0, nn1) * 4096 + nn0 + nn1,
        nn0 * 4096 + nn1,
        nn1 * 4096 + nn0,
        nn0 + nn1,
    ):
        order = np.argsort(-skey, kind="stable")
        tot = 0
        for r in range(SLOTS):
            blk = order[r * M_CORES:(r + 1) * M_CORES]
            tot += int(nn0[blk].max()) + int(nn1[blk].max())
        if best is None or tot < best[0]:
            best = (tot, order)
    order = best[1]
    assign = np.full((M_CORES, SLOTS), -1, dtype=np.int64)
    core_of = np.zeros(NG, dtype=np.int64)
    slot_of = np.zeros(NG, dtype=np.int64)
    for r in range(SLOTS):
        blk = order[r * M_CORES:(r + 1) * M_CORES]
        for c, g in enumerate(blk):
            assign[c, r] = g
            core_of[g] = c
            slot_of[g] = r

    # local search: swap a core's groups between slots when it lowers the
    # summed per-slot maxima (what the gather actually issues)
    def _slot_cost(r):
        blk = assign[:, r]
        blk = blk[blk >= 0]
        return int(nn0[blk].max()) + int(nn1[blk].max())
    cost = np.array([_slot_cost(r) for r in range(SLOTS)], dtype=np.int64)
    for _ in range(6):
        improved = False
        for c in range(M_CORES):
            for r1 in range(SLOTS):
                for r2 in range(r1 + 1, SLOTS):
                    g1, g2 = assign[c, r1], assign[c, r2]
                    if g1 < 0 or g2 < 0:
                        continue
                    assign[c, r1], assign[c, r2] = g2, g1
                    n1, n2 = _slot_cost(r1), _slot_cost(r2)
                    if n1 + n2 < cost[r1] + cost[r2]:
                        cost[r1], cost[r2] = n1, n2
                        improved = True
                    else:
                        assign[c, r1], assign[c, r2] = g1, g2
        if not improved:
            break
    for c in range(M_CORES):
        for r in range(SLOTS):
            g = int(assign[c, r])
            if g >= 0:
                core_of[g] = c
                slot_of[g] = r

    nmax = np.zeros((SLOTS, 2), dtype=np.int64)
    for r in range(SLOTS):
        blk = assign[:, r]
        blk = blk[blk >= 0]
        nmax[r, 0] = int(nn0[blk].max())
        nmax[r, 1] = int(nn1[blk].max())

    # arena packing per half (shared across cores)
    arA, pos0A, aidA = _pack_arenas(nmax[:, 0])
    arB, pos0B, aidB = _pack_arenas(nmax[:, 1])
    # flat idx offsets, each arena padded to a full last chunk: the gather
    # then writes every byte a matmul can read, so no SBUF pre-zeroing (or
    # NaN-bit hazard from uninitialized tails) exists at all
    ist = 0
    for ar in arA + arB:
        ar["istart"] = ist
        ar["nissue"] = ar["nchunks"] * P
        ist += ar["nissue"]
    ITOT = ist

    # per-slot S-column layout: [lenA cols][lenB cols][self]
    loA = np.zeros(SLOTS, dtype=np.int64)
    lenA = np.zeros(SLOTS, dtype=np.int64)
    loB = np.zeros(SLOTS, dtype=np.int64)
    lenB = np.zeros(SLOTS, dtype=np.int64)
    for s in range(SLOTS):
        for (pos0, nm, lo, ln) in ((pos0A, nmax[s, 0], loA, lenA),
                                   (pos0B, nmax[s, 1], loB, lenB)):
            p0 = int(pos0[s])
            n = int(nm)
            if n == 0:
                lo[s] = 0
                ln[s] = 0
            else:
                lo[s] = p0 // P
                ln[s] = -(-(p0 + n) // P) - p0 // P
    scol = np.zeros(SLOTS, dtype=np.int64)
    run = 0
    for s in range(SLOTS):
        scol[s] = run
        run += int(lenA[s]) + int(lenB[s]) + 1
    NCOLS = run

    # ---- per-core value arrays
    eorder = np.argsort(key, kind="stable")
    ks = key[eorder]
    ss = src[eorder]
    nn = nrm[eorder]
    do = doff[eorder]
    starts = np.concatenate([[0], np.cumsum(cnt)[:-1]])
    pos = np.arange(ks.shape[0], dtype=np.int64) - starts[ks]

    g_e = ks // 2
    h_e = ks % 2
    cr_e = core_of[g_e]
    s_e = slot_of[g_e]
    # flat position: arena istart + in-arena slot start + in-slot index
    ar_ist = np.zeros((SLOTS, 2), dtype=np.int64)
    for s in range(SLOTS):
        ar_ist[s, 0] = arA[aidA[s]]["istart"] + pos0A[s]
        ar_ist[s, 1] = arB[aidB[s]]["istart"] + pos0B[s]
    gpos = ar_ist[s_e, h_e] + pos

    idx_arr = np.zeros((M_CORES, ITOT), dtype=np.int16)
    idx_arr[cr_e, gpos] = (ss - h_e * HALF).astype(np.int16)

    # S columns: value arrays [M, NCOLS, P]
    dofcol = np.zeros((M_CORES, NCOLS, P), dtype=np.float32)
    nrmcol = np.zeros((M_CORES, NCOLS, P), dtype=np.float32)
    # in-arena position of each edge
    ipos_e = (pos0A[s_e] * (1 - h_e) + pos0B[s_e] * h_e) + pos
    chunk_e = ipos_e // P
    part_e = ipos_e % P
    colbase = scol[s_e] + np.where(h_e == 0, chunk_e - loA[s_e],
                                   lenA[s_e] + chunk_e - loB[s_e])
    dofcol[cr_e, colbase, part_e] = do
    nrmcol[cr_e, colbase, part_e] = nn

    # self columns + xself/xres slabs
    xbf = x.astype(BF)
    xself_sl = []
    for c in range(M_CORES):
        slab = np.zeros((P, SLOTS, D), dtype=BF)
        for s in range(SLOTS):
            g = int(assign[c, s])
            cself = int(scol[s] + lenA[s] + lenB[s])
            dofcol[c, cself, :] = np.arange(P, dtype=np.float32)
            if g < 0:
                continue
            lo = g * P
            hi = min(lo + P, N_NODES)
            n = hi - lo
            slab[:n, s, :] = xbf[lo:hi]
            nrmcol[c, cself, :n] = dinv2[lo:hi]
        xself_sl.append(np.ascontiguousarray(slab.reshape(P, SLOTS * D)))
    # residual is derived on-device: out = pout + a*xself + (1-a)*b.
    # xres input now carries only the (1-a)*b bias row + the alpha scalar.
    brow = np.tile(((1.0 - a) * bf)[None, :], (P, 1)).astype(np.float32)
    xres_sl = [brow for _ in range(M_CORES)]
    has_bias = bool(np.any(bf != 0.0))

    gidx = [np.tile(idx_arr[c].reshape(-1, 16).T, (8, 1)) for c in range(M_CORES)]
    nrm_in = [np.ascontiguousarray(nrmcol[c].reshape(NCOLS * P)
                                   .reshape(NCOLS, P).T).astype(BF)
              for c in range(M_CORES)]
    off_in = [np.ascontiguousarray(dofcol[c].reshape(NCOLS, P).T).astype(BF)
              for c in range(M_CORES)]

    wtp = np.ascontiguousarray(((1.0 - a) * Wf.T).astype(np.float32))
    CMAXS = int(max(int(lenA[s]) + int(lenB[s]) + 1 for s in range(SLOTS)))
    # iota2[p, (i c)] = i: S is built in [p, i, c] layout so the per-slot
    # dof/nrm operands broadcast along the middle dim (2x DVE rate vs inner)
    iota = np.tile(
        np.broadcast_to(np.arange(P, dtype=np.float32)[:, None],
                        (P, CMAXS)).reshape(1, P * CMAXS),
        (P, 1)).astype(BF)
    ident = np.eye(P, dtype=np.float32)

    meta = dict(arA=arA, arB=arB, loA=loA, lenA=lenA, loB=loB, lenB=lenB,
                scol=scol, NCOLS=NCOLS, ITOT=ITOT, aidA=aidA, aidB=aidB,
                assign=assign, CMAXS=CMAXS, alpha=a, has_bias=has_bias)
    return (xbf, gidx, nrm_in, off_in, xres_sl, xself_sl, wtp, iota, ident, meta)


def _build(meta):
    ALPHA_C = float(meta["alpha"])
    arA, arB = meta["arA"], meta["arB"]
    loA, lenA = meta["loA"], meta["lenA"]
    loB, lenB = meta["loB"], meta["lenB"]
    scol, NCOLS, ITOT = meta["scol"], meta["NCOLS"], meta["ITOT"]
    aidA, aidB = meta["aidA"], meta["aidB"]

    CMAXS = int(meta["CMAXS"])

    nc = bacc.Bacc("TRN2", debug=False, num_swdge_queues=4)

    xtab = nc.dram_tensor("xtab", [N_NODES, D], BF16, kind="ExternalInput")
    xres = nc.dram_tensor("xres", [P, D], F32, kind="ExternalInput")
    xself = nc.dram_tensor("xself", [P, SLOTS * D], BF16, kind="ExternalInput")
    gidx = nc.dram_tensor("gidx", [P, ITOT // 16], I16, kind="ExternalInput")
    nrmv = nc.dram_tensor("nrmv", [P, NCOLS], BF16, kind="ExternalInput")
    dofv = nc.dram_tensor("dofv", [P, NCOLS], BF16, kind="ExternalInput")
    wtp = nc.dram_tensor("wtp", [2 * P, D], F32R, kind="ExternalInput")
    iota = nc.dram_tensor("iota", [P, P * CMAXS], BF16, kind="ExternalInput")
    ident = nc.dram_tensor("ident", [P, P], F32, kind="ExternalInput")
    out = nc.dram_tensor("out", [SLOTS * P, D], F32, kind="ExternalOutput")

    with tile.TileContext(nc) as tc:
        with (
            tc.tile_pool(name="const", bufs=1) as cpool,
            tc.tile_pool(name="arA", bufs=6) as arA_pool,
            tc.tile_pool(name="arB", bufs=6) as arB_pool,
            tc.tile_pool(name="sel", bufs=3) as s_pool,
            tc.tile_pool(name="sb", bufs=3) as sb_pool,
            tc.tile_pool(name="io", bufs=3) as io_pool,
            tc.tile_pool(name="pagg", bufs=2, space="PSUM") as pagg_pool,
            tc.tile_pool(name="pt", bufs=2, space="PSUM") as pt_pool,
            tc.tile_pool(name="pout", bufs=2, space="PSUM") as pout_pool,
        ):
            iota_sb = cpool.tile([P, P, CMAXS], BF16)
            ident_sb = cpool.tile([P, P], F32)
            wtp0_sb = cpool.tile([P, D], F32R)
            wtp1_sb = cpool.tile([P, D], F32R)
            gidx_sb = cpool.tile([P, ITOT // 16], I16)
            nrm_sb = cpool.tile([P, NCOLS], BF16)
            dof_sb = cpool.tile([P, NCOLS], BF16)
            xself_sb = cpool.tile([P, SLOTS, D], BF16)
            g0c = None  # first-arena idx cols load first so gather 0 starts asap
            _g0 = [a for a in (meta["arA"][0], meta["arB"][0])]
            g0c = (_g0[0]["nissue"] + _g0[1]["nissue"]) // 16
            nc.scalar.dma_start(out=gidx_sb[:, 0:g0c], in_=gidx[:, 0:g0c])
            nc.sync.dma_start(out=nrm_sb[:], in_=nrmv[:])
            nc.sync.dma_start(out=dof_sb[:], in_=dofv[:])
            nc.sync.dma_start(
                out=iota_sb[:].rearrange("p i c -> p (i c)"), in_=iota[:])
            nc.sync.dma_start(out=ident_sb[:], in_=ident[:])
            nc.sync.dma_start(out=wtp0_sb[:], in_=wtp[0:P, :])
            nc.sync.dma_start(out=wtp1_sb[:], in_=wtp[P:2 * P, :])
            bias_sb = cpool.tile([P, D], F32)
            nc.sync.dma_start(out=bias_sb[:], in_=xres[:])
            nc.sync.dma_start(out=gidx_sb[:, g0c:], in_=gidx[:, g0c:])
            nc.sync.dma_start(
                out=xself_sb[:].rearrange("p s d -> p (s d)"), in_=xself[:])

            # 16-idx warmup gather: pays the ~6us ext-isa IRAM load while
            # the constant DMAs are still in flight, so the first real
            # gather starts generating immediately.
            warm_idx = cpool.tile([P, 1], I16)
            warm_out = cpool.tile([P, 1, D], BF16)
            nc.vector.memset(warm_idx[:], 0)
            nc.gpsimd.dma_gather(
                warm_out[:], xtab[0:HALF, :], warm_idx[:],
                16, 16, D, single_packet=False, queue_num=0,
            )

            qrr = [0]

            def gather_arena(pool, ar, tab_ap, tag):
                t = pool.tile([P, CAP, D], BF16, tag=tag)
                q = qrr[0] % 4
                qrr[0] += 1
                nc.gpsimd.dma_gather(
                    t[:, 0:ar["nchunks"], :], tab_ap,
                    gidx_sb[:, ar["istart"] // 16:
                            ar["istart"] // 16 + ar["nissue"] // 16],
                    ar["nissue"], ar["nissue"], D,
                    single_packet=False, queue_num=q,
                )
                return t

            tabA = xtab[0:HALF, :]
            tabB = xtab[HALF:N_NODES, :]
            curA = None
            curB = None
            for s in range(SLOTS):
                a_id, b_id = int(aidA[s]), int(aidB[s])
                if arA[a_id]["first_slot"] == s:
                    curA = gather_arena(arA_pool, arA[a_id], tabA, "xga")
                if arB[b_id]["first_slot"] == s:
                    curB = gather_arena(arB_pool, arB[b_id], tabB, "xgb")

                lA, lB = int(lenA[s]), int(lenB[s])
                ncols = lA + lB + 1
                sc = int(scol[s])
                # S in [p, i, c] layout: dof/nrm broadcast along the middle
                # dim (fast); matmul lhsT takes strided [:, :, c] slices.
                s_tile = s_pool.tile([P, P, CMAXS], BF16, tag="sel")
                dof_b = (dof_sb[:, sc:sc + ncols]
                         .rearrange("p (i c) -> p i c", i=1)
                         .to_broadcast([P, P, ncols]))
                nrm_b = (nrm_sb[:, sc:sc + ncols]
                         .rearrange("p (i c) -> p i c", i=1)
                         .to_broadcast([P, P, ncols]))
                nc.vector.tensor_tensor(
                    out=s_tile[:, :, 0:ncols], in0=iota_sb[:, :, 0:ncols],
                    in1=dof_b, op=mybir.AluOpType.is_equal,
                )
                nc.vector.tensor_tensor(
                    out=s_tile[:, :, 0:ncols], in0=s_tile[:, :, 0:ncols],
                    in1=nrm_b, op=mybir.AluOpType.mult,
                )

                pagg = pagg_pool.tile([P, D], F32)
                k = 0
                for j in range(lA):
                    nc.tensor.matmul(
                        pagg[:], lhsT=s_tile[:, :, j],
                        rhs=curA[:, int(loA[s]) + j, :],
                        start=(k == 0), stop=False)
                    k += 1
                for j in range(lB):
                    nc.tensor.matmul(
                        pagg[:], lhsT=s_tile[:, :, lA + j],
                        rhs=curB[:, int(loB[s]) + j, :],
                        start=(k == 0), stop=False)
                    k += 1
                nc.tensor.matmul(
                    pagg[:], lhsT=s_tile[:, :, lA + lB],
                    rhs=xself_sb[:, s, :], start=(k == 0), stop=True)

                agg_sb = sb_pool.tile([P, D], F32, tag="agg")
                nc.scalar.copy(agg_sb[:], pagg[:])

                aggT_sb = sb_pool.tile([P, D], F32R, tag="aggT")
                for kb in range(2):
                    pt = pt_pool.tile([P, P], F32)
                    nc.tensor.transpose(
                        pt[:], agg_sb[:, kb * P:(kb + 1) * P], ident_sb[:]
                    )
                    nc.scalar.copy(aggT_sb[:, kb * P:(kb + 1) * P], pt[:])

                pout = pout_pool.tile([P, D], F32)
                nc.tensor.matmul(
                    pout[:], lhsT=aggT_sb[:, 0:P],
                    rhs=wtp0_sb[:], start=True, stop=False,
                )
                nc.tensor.matmul(
                    pout[:], lhsT=aggT_sb[:, P:2 * P],
                    rhs=wtp1_sb[:], start=False, stop=True,
                )

                out_sb = io_pool.tile([P, D], F32, tag="out")
                # out = a*xself + pout  (alpha from bias_sb[1,0] at build is a
                # host constant folded via scalar_tensor_tensor's scalar)
                nc.vector.scalar_tensor_tensor(
                    out=out_sb[:], in0=xself_sb[:, s, :], scalar=ALPHA_C,
                    in1=pout[:], op0=mybir.AluOpType.mult,
                    op1=mybir.AluOpType.add,
                )
                if meta["has_bias"]:
                    nc.vector.tensor_tensor(
                        out=out_sb[:], in0=out_sb[:], in1=bias_sb[:],
                        op=mybir.AluOpType.add,
                    )
                nc.sync.dma_start(out=out[s * P:(s + 1) * P, :], in_=out_sb[:])

    nc.compile()
    return nc


def kernel(node_features, edge_index, W, b, alpha):
    (x, gidx, nrm_in, off_in, xres_sl, xself_sl, wtp, iota, ident, meta) = _preprocess(
        node_features, edge_index, W, b, alpha
    )
    nc = _build(meta)
    in_maps = [
        {
            "xtab": x,
            "xres": xres_sl[c],
            "xself": xself_sl[c],
            "gidx": gidx[c],
            "nrmv": nrm_in[c],
            "dofv": off_in[c],
            "wtp": wtp,
            "iota": iota,
            "ident": ident,
        }
        for c in range(M_CORES)
    ]
    res = run_bass_kernel_spmd(nc, in_maps, list(range(M_CORES)))
    assign = meta["assign"]
    outf = np.empty((N_NODES, D), dtype=np.float32)
    for c in range(M_CORES):
        slab = res.results[c]["out"]
        for s in range(SLOTS):
            g = int(assign[c, s])
            if g < 0:
                continue
            lo = g * P
            hi = min(lo + P, N_NODES)
            outf[lo:hi] = slab[s * P: s * P + hi - lo]
    return outf
